# revision 18
# baseline (speedup 1.0000x reference)
"""Trainium2 Bass kernel for nn_BackpropagationBasedFiltering.

Computation (see reference):
  L = exp(lines)[bone_idx]                       (49,)
  norm = sqrt(ax^2+ay^2+az^2) + 1e-10            (T,49)
  d{x,y,z} = L * a{x,y,z} / norm                 (T,49)
  x = rootsx + dx @ PATH_T                       (T,50)   (binary-tree path sums)
  loss = sum(w*(x-tx)^2 + w*(y-ty)^2) / (T*50)
  reg1 = sum(exp(lines))
  reg2 = sum((x[:-1]-x[1:])^2 + ... y,z ...) / ((T-1)*50)
  returns (x, y, z, loss + 0.001*reg1 + 0.1*reg2)

Strategy: data-parallel over T on 8 cores, 24960 rows/core (128 partitions x
stripes of 195 consecutive rows). The path-matrix product is evaluated as a
level-vectorized binary-tree prefix sum on the vector engine with strided
access patterns (no PE transposes needed). Loss/reg2 reduced on-chip
(ACT square+accum, DVE tensor_tensor_reduce); scalar partials combined on
host. The last 320 rows and the stripe/core-boundary reg2 pairs are computed
on the host (0.16% of the work).
"""

import sys

sys.path.insert(0, "/opt/trn_rl_repo")

import numpy as np

import bass_rust
import concourse.bass as bass
import concourse.tile as tile_mod
from concourse import mybir
from concourse.bass_utils import run_bass_kernel_spmd
from concourse.tile import TileContext


def _drain_and_barrier_split(self, tick_clock, wait_clock):
    """Tail drain with one sem-wait per instruction: the walrus build here
    rejects CTRL instructions carrying >4 sync waits, and Tile's stock tail
    drain waits on every tracked semaphore at once."""
    nc = self.nc
    drain_inst = nc.sync.drain()
    wait_clock.add_sem_waits(
        drain_inst.ins, tile_mod.ScopedClock({None: tick_clock.global_clock})
    )
    si = drain_inst.ins.sync_info
    waits = list(si.on_wait or [])
    if len(waits) > 1:
        si.on_wait = waits[:1]
        for w in waits[1:]:
            nop = nc.sync.nop(nofuse=True, hint="tail_wait_split")
            nsi = nop.ins.sync_info
            if nsi is None:
                nop.ins.sync_info = bass_rust.SyncInfo(on_wait=[w], on_update=[])
            else:
                nsi.on_wait = [w]
    nc.all_engine_barrier()
    popped = nc._tile_sem_poison_stack.pop()
    assert popped is self._sem_poison
    nc.clear_and_free_semaphores(list(self.sems.allocated().values()))
    nc.all_engine_barrier()


TileContext._drain_and_barrier = _drain_and_barrier_split

_MAX_WAITS = 1  # this walrus rejects instructions with more sync waits


def _split_excess_waits(nc):
    """Hoist excess sem-waits onto same-engine NOPs emitted just before the
    offending instruction (engines execute their stream in order, so the
    semantics are identical)."""
    uid = [0]
    for f in nc.m.functions:
        for bb in f.blocks:
            insts = bb.instructions
            rebuilt = []
            changed = False
            for inst in insts:
                si = getattr(inst, "sync_info", None)
                ow = list(si.on_wait) if si is not None and si.on_wait else []
                if len(ow) > _MAX_WAITS:
                    keep = ow[-_MAX_WAITS:]
                    excess = ow[:-_MAX_WAITS]
                    for j in range(0, len(excess), _MAX_WAITS):
                        nop = mybir.InstNoOp(
                            name=f"waitsplit-{uid[0]}", ins=[], outs=[])
                        uid[0] += 1
                        nop.engine = inst.engine
                        nop.sync_info = bass_rust.SyncInfo(
                            on_wait=excess[j:j + _MAX_WAITS], on_update=[])
                        rebuilt.append(nop)
                    si.on_wait = keep
                    changed = True
                rebuilt.append(inst)
            if changed:
                del insts[:]
                for inst in rebuilt:
                    insts.append(inst)

# ---------------------------------------------------------------- constants
T = 200000
NL = 49          # limbs
NP = 50          # points
NB = 20          # bones
EPS = 1e-10
REG_RATES = (0.001, 0.1)
NCORES = 8
P = 128          # partitions
S = 195          # stripe length (rows per partition per core)
RPC = P * S      # rows per core = 24960
MAIN = NCORES * RPC  # 199680
# groups of tiles processed per instruction batch (sum == S)
G_LIST = [33, 33, 33, 32, 32, 32]
NGRP = len(G_LIST)

STRUCTURE = [((b - 1) // 2, b, (b - 1) % NB) for b in range(1, NP)]
BONE_IDX = np.array([l for (_, _, l) in STRUCTURE], dtype=np.int64)
# tree levels: (lo, hi) node ranges; children of p are 2p+1, 2p+2
LEVELS = [(1, 3), (3, 7), (7, 15), (15, 31), (31, 50)]

F32 = mybir.dt.float32
BF16 = mybir.dt.bfloat16

TRACE = False  # test harness can flip this to collect an NTFF profile
_cache = {}


def _path_matrix():
    Pm = np.zeros((NP, NL), dtype=np.float32)
    for i, (a, b, _) in enumerate(STRUCTURE):
        Pm[b] = Pm[a]
        Pm[b, i] = 1.0
    return Pm


def _ap3(tile, col_off, dims):
    """AP into `tile` (full partition dim) with custom free dims, offset in
    elements from the tile's first column."""
    t = tile[:]
    return bass.AP(tensor=t.tensor, offset=t.offset + col_off, ap=[t.ap[0]] + dims)


def _build_bass(stripe=S, g_list=G_LIST, split_waits=True, epochs=1):
    """Build the per-core bass program. All 8 cores run it SPMD on their
    own 24960-row slice."""
    nc = bass.Bass()
    rows = P * stripe
    ngrp = len(g_list)

    ax_e = nc.dram_tensor("ax", [rows, NL], F32, kind="ExternalInput")
    ay_e = nc.dram_tensor("ay", [rows, NL], F32, kind="ExternalInput")
    az_e = nc.dram_tensor("az", [rows, NL], F32, kind="ExternalInput")
    tx_e = nc.dram_tensor("tx", [rows, NP], F32, kind="ExternalInput")
    ty_e = nc.dram_tensor("ty", [rows, NP], F32, kind="ExternalInput")
    w_e = nc.dram_tensor("w", [rows, NP], F32, kind="ExternalInput")
    rx_e = nc.dram_tensor("rx", [rows, 1], F32, kind="ExternalInput")
    ry_e = nc.dram_tensor("ry", [rows, 1], F32, kind="ExternalInput")
    rz_e = nc.dram_tensor("rz", [rows, 1], F32, kind="ExternalInput")
    lt_e = nc.dram_tensor("ltile", [1, max(g_list) * NL], F32, kind="ExternalInput")

    x_o = nc.dram_tensor("x", [rows, NP], F32, kind="ExternalOutput")
    y_o = nc.dram_tensor("y", [rows, NP], F32, kind="ExternalOutput")
    z_o = nc.dram_tensor("z", [rows, NP], F32, kind="ExternalOutput")
    sl_o = nc.dram_tensor("sloss", [P, 2 * ngrp], F32, kind="ExternalOutput")
    sr_o = nc.dram_tensor("sreg2", [P, 3 * ngrp], F32, kind="ExternalOutput")

    # DRAM views: row (p*stripe + n)  ->  partition p, tile n
    axv = ax_e[:].rearrange("(p s) l -> p s l", p=P)
    ayv = ay_e[:].rearrange("(p s) l -> p s l", p=P)
    azv = az_e[:].rearrange("(p s) l -> p s l", p=P)
    txv = tx_e[:].rearrange("(p s) l -> p s l", p=P)
    tyv = ty_e[:].rearrange("(p s) l -> p s l", p=P)
    wv = w_e[:].rearrange("(p s) l -> p s l", p=P)
    xv = x_o[:].rearrange("(p s) l -> p s l", p=P)
    yv = y_o[:].rearrange("(p s) l -> p s l", p=P)
    zv = z_o[:].rearrange("(p s) l -> p s l", p=P)
    rxv = rx_e[:].rearrange("(p s) 1 -> p s", p=P)
    ryv = ry_e[:].rearrange("(p s) 1 -> p s", p=P)
    rzv = rz_e[:].rearrange("(p s) 1 -> p s", p=P)

    gmax = max(g_list)
    ADD = mybir.AluOpType.add
    SUB = mybir.AluOpType.subtract
    MUL = mybir.AluOpType.mult

    with TileContext(nc) as tc:
        with (
            tc.tile_pool(name="pa", bufs=2) as pa,
            tc.tile_pool(name="ptar", bufs=2) as ptar,
            tc.tile_pool(name="pout", bufs=2) as pout,
            tc.tile_pool(name="psq", bufs=2) as psq,
            tc.tile_pool(name="pscr", bufs=2) as pscr,
            tc.tile_pool(name="pone", bufs=1) as pone,
        ):
            # one-time loads
            lt = pone.tile([P, gmax * NL], F32, tag="lt")
            ltap = lt_e[:]
            nc.gpsimd.dma_start(
                out=lt[:],
                in_=bass.AP(tensor=ltap.tensor, offset=ltap.offset,
                            ap=[[0, P], [1, gmax * NL]]),
            )
            r_sb = []
            for rv, tag in ((rxv, "rx"), (ryv, "ry"), (rzv, "rz")):
                rt = pone.tile([P, stripe], F32, tag=tag)
                nc.sync.dma_start(out=rt[:], in_=rv)
                r_sb.append(rt)
            acc_l = pone.tile([P, 2 * ngrp], F32, tag="accl")
            acc_r = pone.tile([P, 3 * ngrp], F32, tag="accr")

            for _ep in range(epochs):
              g0 = 0
              for gi, G in enumerate(g_list):
                W49 = G * NL
                W50 = G * NP
                # ---- loads
                a_t = []
                for v, tag in ((axv, "ax"), (ayv, "ay"), (azv, "az")):
                    at = pa.tile([P, W49], F32, tag=tag)
                    nc.sync.dma_start(out=at[:], in_=v[:, g0:g0 + G, :])
                    a_t.append(at)
                tx_t = ptar.tile([P, W50], F32, tag="tx")
                nc.sync.dma_start(out=tx_t[:], in_=txv[:, g0:g0 + G, :])
                ty_t = ptar.tile([P, W50], F32, tag="ty")
                nc.sync.dma_start(out=ty_t[:], in_=tyv[:, g0:g0 + G, :])
                w_t = ptar.tile([P, W50], F32, tag="w")
                nc.sync.dma_start(out=w_t[:], in_=wv[:, g0:g0 + G, :])

                # ---- norm: s = ax^2+ay^2+az^2 ; inv = L / (sqrt(s))
                s_t = psq.tile([P, W49], F32, tag="s")
                q_t = psq.tile([P, W49], F32, tag="q")
                nc.scalar.square(s_t[:], a_t[0][:])
                nc.scalar.square(q_t[:], a_t[1][:])
                nc.vector.tensor_tensor(s_t[:], s_t[:], q_t[:], ADD)
                nc.scalar.square(q_t[:], a_t[2][:])
                nc.vector.tensor_tensor(s_t[:], s_t[:], q_t[:], ADD)
                # 1/sqrt(s) = exp(-0.5*log(s)); ACT Rsqrt/Reciprocal are
                # banned for accuracy and Log/Exp share one table set.
                nc.scalar.activation(s_t[:], s_t[:],
                                     mybir.ActivationFunctionType.Ln)
                nc.scalar.activation(s_t[:], s_t[:],
                                     mybir.ActivationFunctionType.Exp,
                                     scale=-0.5)
                nc.vector.tensor_tensor(s_t[:], s_t[:], lt[:, :W49], MUL)
                # d = a * inv  (in place over the angle tiles)
                for at in a_t:
                    nc.vector.tensor_tensor(at[:], at[:], s_t[:], MUL)

                # ---- tree prefix sums -> x,y,z  (out tiles, layout [P, G, 50])
                o_t = []
                for (dt_, rt, tag) in ((a_t[0], r_sb[0], "xo"),
                                       (a_t[1], r_sb[1], "yo"),
                                       (a_t[2], r_sb[2], "zo")):
                    ot = pout.tile([P, W50], F32, tag=tag)
                    # root column := roots
                    nc.vector.tensor_copy(
                        _ap3(ot, 0, [[NP, G], [1, 1]]),
                        _ap3(rt, g0, [[1, G], [1, 1]]),
                    )
                    for (lo, hi) in LEVELS:
                        n = hi - lo
                        npair = n // 2
                        plo = (lo - 1) // 2
                        if npair:
                            nc.vector.tensor_tensor(
                                _ap3(ot, lo, [[NP, G], [2, npair], [1, 2]]),
                                _ap3(ot, plo, [[NP, G], [1, npair], [0, 2]]),
                                _ap3(dt_, lo - 1, [[NL, G], [2, npair], [1, 2]]),
                                ADD,
                            )
                        if n % 2:  # leftover last node (49 <- parent 24, limb 48)
                            b = hi - 1
                            nc.vector.tensor_tensor(
                                _ap3(ot, b, [[NP, G], [1, 1]]),
                                _ap3(ot, (b - 1) // 2, [[NP, G], [1, 1]]),
                                _ap3(dt_, b - 1, [[NL, G], [1, 1]]),
                                ADD,
                            )
                    o_t.append(ot)

                # ---- store x,y,z
                nc.scalar.dma_start(out=xv[:, g0:g0 + G, :], in_=o_t[0][:])
                nc.scalar.dma_start(out=yv[:, g0:g0 + G, :], in_=o_t[1][:])
                nc.scalar.dma_start(out=zv[:, g0:g0 + G, :], in_=o_t[2][:])

                # ---- loss partials: sum w*(x-t)^2 over the group
                for ci, (ot, tt) in enumerate(((o_t[0], tx_t), (o_t[1], ty_t))):
                    ex = pscr.tile([P, W50], F32, tag="scr_a")
                    nc.vector.tensor_tensor(ex[:], ot[:], tt[:], SUB)
                    ex2 = pscr.tile([P, W50], F32, tag="scr_b")
                    nc.scalar.square(ex2[:], ex[:])
                    exw = pscr.tile([P, W50], F32, tag="scr_j")
                    nc.vector.tensor_tensor(exw[:], ex2[:], w_t[:], MUL)
                    nc.vector.tensor_reduce(
                        acc_l[:, 2 * gi + ci: 2 * gi + ci + 1],
                        exw[:],
                        mybir.AxisListType.X,
                        ADD,
                    )

                # ---- reg2 partials: sum (x[n]-x[n+1])^2 within the group
                Wd = (G - 1) * NP
                for ci, ot in enumerate(o_t):
                    df = pscr.tile([P, Wd], F32, tag="scr_a")
                    nc.vector.tensor_tensor(
                        _ap3(df, 0, [[NP, G - 1], [1, NP]]),
                        _ap3(ot, 0, [[NP, G - 1], [1, NP]]),
                        _ap3(ot, NP, [[NP, G - 1], [1, NP]]),
                        SUB,
                    )
                    df2 = pscr.tile([P, Wd], BF16, tag="scr_j")
                    nc.scalar.activation(
                        out=df2[:],
                        in_=df[:],
                        func=mybir.ActivationFunctionType.Square,
                        accum_out=acc_r[:, 3 * gi + ci: 3 * gi + ci + 1],
                    )
                g0 += G

            nc.sync.dma_start(out=sl_o[:], in_=acc_l[:])
            nc.sync.dma_start(out=sr_o[:], in_=acc_r[:])

    if split_waits:
        _split_excess_waits(nc)
    return nc


def _get_nc():
    if "nc" not in _cache:
        _cache["nc"] = _build_bass()
    return _cache["nc"]


def _host_ref(lines, rx, ry, rz, ax, ay, az):
    """Host fallback math for a row slab: returns x, y, z (f32)."""
    L = np.exp(lines.astype(np.float64))[BONE_IDX]
    na = np.sqrt(ax.astype(np.float64) ** 2 + ay.astype(np.float64) ** 2
                 + az.astype(np.float64) ** 2) + EPS
    Pm = _path_matrix().astype(np.float64)
    dx = L * ax / na
    dy = L * ay / na
    dz = L * az / na
    x = rx + dx @ Pm.T
    y = ry + dy @ Pm.T
    z = rz + dz @ Pm.T
    return x.astype(np.float32), y.astype(np.float32), z.astype(np.float32)


def _missing_pairs():
    """reg2 pairs (r, r+1) not covered on-device."""
    if "mp" in _cache:
        return _cache["mp"]
    covered = np.zeros(T - 1, dtype=bool)
    ns = []
    g0 = 0
    for G in G_LIST:
        ns.append(np.arange(g0, g0 + G - 1))
        g0 += G
    ns = np.concatenate(ns)  # within-stripe tile indices with covered pair
    base = (np.arange(NCORES)[:, None] * RPC
            + np.arange(P)[None, :] * S).reshape(-1, 1)
    rows = (base + ns[None, :]).ravel()
    covered[rows] = True
    miss = np.nonzero(~covered)[0]
    _cache["mp"] = miss
    return miss


def kernel(**inputs):
    lines = np.asarray(inputs["lines"], dtype=np.float32)
    rootsx = np.asarray(inputs["rootsx"], dtype=np.float32)
    rootsy = np.asarray(inputs["rootsy"], dtype=np.float32)
    rootsz = np.asarray(inputs["rootsz"], dtype=np.float32)
    anglesx = np.asarray(inputs["anglesx"], dtype=np.float32)
    anglesy = np.asarray(inputs["anglesy"], dtype=np.float32)
    anglesz = np.asarray(inputs["anglesz"], dtype=np.float32)
    tarx = np.asarray(inputs["tarx_values"], dtype=np.float32)
    tary = np.asarray(inputs["tary_values"], dtype=np.float32)
    wv = np.asarray(inputs["w_values"], dtype=np.float32)

    L_limb = np.exp(lines)[BONE_IDX]  # (49,) f32
    ltile = np.ascontiguousarray(
        np.tile(L_limb, max(G_LIST))[None, :]).astype(np.float32)

    nc = _get_nc()
    in_maps = []
    for k in range(NCORES):
        a, b = k * RPC, (k + 1) * RPC
        in_maps.append({
            "ax": np.ascontiguousarray(anglesx[a:b]),
            "ay": np.ascontiguousarray(anglesy[a:b]),
            "az": np.ascontiguousarray(anglesz[a:b]),
            "tx": np.ascontiguousarray(tarx[a:b]),
            "ty": np.ascontiguousarray(tary[a:b]),
            "w": np.ascontiguousarray(wv[a:b]),
            "rx": np.ascontiguousarray(rootsx[a:b]),
            "ry": np.ascontiguousarray(rootsy[a:b]),
            "rz": np.ascontiguousarray(rootsz[a:b]),
            "ltile": ltile,
        })

    res = run_bass_kernel_spmd(nc, in_maps, core_ids=list(range(NCORES)),
                               trace=TRACE)
    _cache["last"] = res
    outs = res.results

    x = np.empty((T, NP), dtype=np.float32)
    y = np.empty((T, NP), dtype=np.float32)
    z = np.empty((T, NP), dtype=np.float32)
    loss_sum = 0.0
    reg2_sum = 0.0
    for k in range(NCORES):
        a, b = k * RPC, (k + 1) * RPC
        x[a:b] = outs[k]["x"]
        y[a:b] = outs[k]["y"]
        z[a:b] = outs[k]["z"]
        loss_sum += float(outs[k]["sloss"].astype(np.float64).sum())
        reg2_sum += float(outs[k]["sreg2"].astype(np.float64).sum())

    # host tail (last 320 rows)
    if MAIN < T:
        sl = slice(MAIN, T)
        xt, yt, zt = _host_ref(lines, rootsx[sl], rootsy[sl], rootsz[sl],
                               anglesx[sl], anglesy[sl], anglesz[sl])
        x[sl], y[sl], z[sl] = xt, yt, zt
        ex = x[sl].astype(np.float64) - tarx[sl]
        ey = y[sl].astype(np.float64) - tary[sl]
        loss_sum += float((wv[sl] * (ex * ex + ey * ey)).sum())

    # host reg2 for pairs not covered on-device
    miss = _missing_pairs()
    for arr in (x, y, z):
        d = arr[miss].astype(np.float64) - arr[miss + 1]
        reg2_sum += float((d * d).sum())

    reg1 = float(np.exp(lines.astype(np.float64)).sum())
    total = (loss_sum / (T * NP) + REG_RATES[0] * reg1
             + REG_RATES[1] * reg2_sum / ((T - 1) * NP))
    return x, y, z, np.float32(total)


# revision 33
# speedup vs baseline: 1.2639x; 1.2639x over previous
"""Trainium2 Bass kernel for nn_BackpropagationBasedFiltering.

Computation (see reference):
  L = exp(lines)[bone_idx]                       (49,)
  norm = sqrt(ax^2+ay^2+az^2) + 1e-10            (T,49)
  d{x,y,z} = L * a{x,y,z} / norm                 (T,49)
  x = rootsx + dx @ PATH_T                       (T,50)   (binary-tree path sums)
  loss = sum(w*(x-tx)^2 + w*(y-ty)^2) / (T*50)
  reg1 = sum(exp(lines))
  reg2 = sum((x[:-1]-x[1:])^2 + ... y,z ...) / ((T-1)*50)
  returns (x, y, z, loss + 0.001*reg1 + 0.1*reg2)

Strategy: data-parallel over T on 8 cores, 24960 rows/core (128 partitions x
stripes of 195 consecutive rows). The path-matrix product is evaluated as a
level-vectorized binary-tree prefix sum on the vector engine with strided
access patterns (no PE transposes needed). Loss/reg2 reduced on-chip
(ACT square+accum, DVE tensor_tensor_reduce); scalar partials combined on
host. The last 320 rows and the stripe/core-boundary reg2 pairs are computed
on the host (0.16% of the work).
"""

import sys

sys.path.insert(0, "/opt/trn_rl_repo")

import numpy as np

import bass_rust
import concourse.bass as bass
import concourse.tile as tile_mod
from concourse import mybir
from concourse.bass_utils import run_bass_kernel_spmd
from concourse.tile import TileContext


def _drain_and_barrier_split(self, tick_clock, wait_clock):
    """Tail drain with one sem-wait per instruction: the walrus build here
    rejects CTRL instructions carrying >4 sync waits, and Tile's stock tail
    drain waits on every tracked semaphore at once."""
    nc = self.nc
    drain_inst = nc.sync.drain()
    wait_clock.add_sem_waits(
        drain_inst.ins, tile_mod.ScopedClock({None: tick_clock.global_clock})
    )
    si = drain_inst.ins.sync_info
    waits = list(si.on_wait or [])
    if len(waits) > 1:
        si.on_wait = waits[:1]
        for w in waits[1:]:
            nop = nc.sync.nop(nofuse=True, hint="tail_wait_split")
            nsi = nop.ins.sync_info
            if nsi is None:
                nop.ins.sync_info = bass_rust.SyncInfo(on_wait=[w], on_update=[])
            else:
                nsi.on_wait = [w]
    nc.all_engine_barrier()
    popped = nc._tile_sem_poison_stack.pop()
    assert popped is self._sem_poison
    nc.clear_and_free_semaphores(list(self.sems.allocated().values()))
    nc.all_engine_barrier()


TileContext._drain_and_barrier = _drain_and_barrier_split

_MAX_WAITS = 1  # this walrus rejects instructions with more sync waits


def _split_excess_waits(nc):
    """Hoist excess sem-waits onto same-engine NOPs emitted just before the
    offending instruction (engines execute their stream in order, so the
    semantics are identical)."""
    uid = [0]
    for f in nc.m.functions:
        for bb in f.blocks:
            insts = bb.instructions
            rebuilt = []
            changed = False
            for inst in insts:
                si = getattr(inst, "sync_info", None)
                ow = list(si.on_wait) if si is not None and si.on_wait else []
                if len(ow) > _MAX_WAITS:
                    keep = ow[-_MAX_WAITS:]
                    excess = ow[:-_MAX_WAITS]
                    for j in range(0, len(excess), _MAX_WAITS):
                        nop = mybir.InstNoOp(
                            name=f"waitsplit-{uid[0]}", ins=[], outs=[])
                        uid[0] += 1
                        nop.engine = inst.engine
                        nop.sync_info = bass_rust.SyncInfo(
                            on_wait=excess[j:j + _MAX_WAITS], on_update=[])
                        rebuilt.append(nop)
                    si.on_wait = keep
                    changed = True
                rebuilt.append(inst)
            if changed:
                del insts[:]
                for inst in rebuilt:
                    insts.append(inst)

# ---------------------------------------------------------------- constants
T = 200000
NL = 49          # limbs
NP = 50          # points
NB = 20          # bones
EPS = 1e-10
REG_RATES = (0.001, 0.1)
NCORES = 8
P = 128          # partitions
S = 195          # stripe length (rows per partition per core)
RPC = P * S      # rows per core = 24960
MAIN = NCORES * RPC  # 199680
# groups of tiles processed per instruction batch (sum == S)
G_LIST = [33, 33, 33, 32, 32, 32]
NGRP = len(G_LIST)

STRUCTURE = [((b - 1) // 2, b, (b - 1) % NB) for b in range(1, NP)]
BONE_IDX = np.array([l for (_, _, l) in STRUCTURE], dtype=np.int64)
# tree levels: (lo, hi) node ranges; children of p are 2p+1, 2p+2
LEVELS = [(1, 3), (3, 7), (7, 15), (15, 31), (31, 50)]

F32 = mybir.dt.float32
BF16 = mybir.dt.bfloat16

TRACE = False  # test harness can flip this to collect an NTFF profile
_cache = {}


def _path_matrix():
    Pm = np.zeros((NP, NL), dtype=np.float32)
    for i, (a, b, _) in enumerate(STRUCTURE):
        Pm[b] = Pm[a]
        Pm[b, i] = 1.0
    return Pm


def _ap3(tile, col_off, dims):
    """AP into `tile` (full partition dim) with custom free dims, offset in
    elements from the tile's first column."""
    t = tile[:]
    return bass.AP(tensor=t.tensor, offset=t.offset + col_off, ap=[t.ap[0]] + dims)


def _build_bass(stripe=S, g_list=G_LIST, split_waits=True, epochs=1,
                z_on_pool=True, diff_on_pool=True, sub_on_pool=True):
    """Build the per-core bass program. All 8 cores run it SPMD on their
    own 24960-row slice.

    Combined-tile layout: the three coordinates live in one SBUF tile
    (coord-major) so each elementwise/tree step is a single wide
    instruction with strided access patterns. Loss uses bf16 scratch with
    host-precomputed sqrt(w); reductions ride the ACT square's accum_out.
    """
    nc = bass.Bass()
    rows = P * stripe
    ngrp = len(g_list)

    ax_e = nc.dram_tensor("ax", [rows, NL], F32, kind="ExternalInput")
    ay_e = nc.dram_tensor("ay", [rows, NL], F32, kind="ExternalInput")
    az_e = nc.dram_tensor("az", [rows, NL], F32, kind="ExternalInput")
    txy_e = nc.dram_tensor("txy", [rows, 2 * NP], BF16, kind="ExternalInput")
    sw_e = nc.dram_tensor("sw", [rows, NP], BF16, kind="ExternalInput")
    rx_e = nc.dram_tensor("rx", [rows, 1], F32, kind="ExternalInput")
    ry_e = nc.dram_tensor("ry", [rows, 1], F32, kind="ExternalInput")
    rz_e = nc.dram_tensor("rz", [rows, 1], F32, kind="ExternalInput")
    lt_e = nc.dram_tensor("ltile", [1, max(g_list) * NL], F32, kind="ExternalInput")

    x_o = nc.dram_tensor("x", [rows, NP], F32, kind="ExternalOutput")
    y_o = nc.dram_tensor("y", [rows, NP], F32, kind="ExternalOutput")
    z_o = nc.dram_tensor("z", [rows, NP], F32, kind="ExternalOutput")
    sl_o = nc.dram_tensor("sloss", [P, ngrp], F32, kind="ExternalOutput")
    sr_o = nc.dram_tensor("sreg2", [1, 512], F32, kind="ExternalOutput")

    # DRAM views: row (p*stripe + n)  ->  partition p, tile n
    axv = ax_e[:].rearrange("(p s) l -> p s l", p=P)
    ayv = ay_e[:].rearrange("(p s) l -> p s l", p=P)
    azv = az_e[:].rearrange("(p s) l -> p s l", p=P)
    txyv = txy_e[:].rearrange("(p s) l -> p s l", p=P)
    swv = sw_e[:].rearrange("(p s) l -> p s l", p=P)
    xv = x_o[:].rearrange("(p s) l -> p s l", p=P)
    yv = y_o[:].rearrange("(p s) l -> p s l", p=P)
    zv = z_o[:].rearrange("(p s) l -> p s l", p=P)
    rxv = rx_e[:].rearrange("(p s) 1 -> p s", p=P)
    ryv = ry_e[:].rearrange("(p s) 1 -> p s", p=P)
    rzv = rz_e[:].rearrange("(p s) 1 -> p s", p=P)

    gmax = max(g_list)
    ADD = mybir.AluOpType.add
    SUB = mybir.AluOpType.subtract
    MUL = mybir.AluOpType.mult
    SQ = mybir.ActivationFunctionType.Square

    with TileContext(nc) as tc:
        with (
            tc.tile_pool(name="pa", bufs=2) as pa,
            tc.tile_pool(name="ptar", bufs=2) as ptar,
            tc.tile_pool(name="pout", bufs=2) as pout,
            tc.tile_pool(name="psq", bufs=2) as psq,
            tc.tile_pool(name="ps2", bufs=3) as ps2,
            tc.tile_pool(name="pscr", bufs=2) as pscr,
            tc.tile_pool(name="pone", bufs=1) as pone,
            tc.tile_pool(name="pps", bufs=1, space="PSUM") as pps,
        ):
            # one-time loads
            lt = pone.tile([P, gmax * NL], F32, tag="lt")
            ltap = lt_e[:]
            nc.gpsimd.dma_start(
                out=lt[:],
                in_=bass.AP(tensor=ltap.tensor, offset=ltap.offset,
                            ap=[[0, P], [1, gmax * NL]]),
            )
            rall = pone.tile([P, 3 * stripe], F32, tag="rall")
            for ci, rv in enumerate((rxv, ryv, rzv)):
                nc.sync.dma_start(out=rall[:, ci * stripe:(ci + 1) * stripe],
                                  in_=rv)
            acc_l = pone.tile([P, ngrp], F32, tag="accl")
            ones_bf = pone.tile([P, 1], BF16, tag="ones")
            nc.vector.memset(ones_bf[:], 1.0)
            rps = pps.tile([1, 512], F32, tag="rps")
            total_mm = epochs * sum(-(-(3 * (G - 1) * NP) // 512)
                                    for G in g_list)  # padded chunks per group
            mm_idx = [0]

            for _ep in range(epochs):
              g0 = 0
              for gi, G in enumerate(g_list):
                W49 = G * NL
                W50 = G * NP
                # ---- loads
                a_all = pa.tile([P, 3 * W49], F32, tag="a")
                for ci, v in enumerate((axv, ayv, azv)):
                    nc.sync.dma_start(out=a_all[:, ci * W49:(ci + 1) * W49],
                                      in_=v[:, g0:g0 + G, :])
                txy_t = ptar.tile([P, 2 * W50], BF16, tag="txy")
                nc.sync.dma_start(out=txy_t[:], in_=txyv[:, g0:g0 + G, :])
                sw_t = ptar.tile([P, W50], BF16, tag="sw")
                nc.sync.dma_start(out=sw_t[:], in_=swv[:, g0:g0 + G, :])

                # ---- norm: s = ax^2+ay^2+az^2 ; inv2 = L * s^-0.5
                sq_all = psq.tile([P, 3 * W49], F32, tag="sq")
                nc.scalar.square(sq_all[:], a_all[:])
                s_t = ps2.tile([P, W49], F32, tag="s")
                nc.vector.tensor_tensor(s_t[:], sq_all[:, :W49],
                                        sq_all[:, W49:2 * W49], ADD)
                nc.vector.tensor_tensor(s_t[:], s_t[:],
                                        sq_all[:, 2 * W49:], ADD)
                # 1/sqrt(s) = exp(-0.5*ln(s)); ACT Rsqrt/Reciprocal are
                # banned for accuracy; Ln/Exp share one table set.
                nc.scalar.activation(s_t[:], s_t[:],
                                     mybir.ActivationFunctionType.Ln)
                nc.scalar.activation(s_t[:], s_t[:],
                                     mybir.ActivationFunctionType.Exp,
                                     scale=-0.5)
                nc.vector.tensor_tensor(s_t[:], s_t[:], lt[:, :W49], MUL)
                # d = a * inv2 (in place; broadcast inv2 across coords)
                nc.vector.tensor_tensor(
                    _ap3(a_all, 0, [[W49, 2], [1, W49]]),
                    _ap3(a_all, 0, [[W49, 2], [1, W49]]),
                    _ap3(s_t, 0, [[0, 2], [1, W49]]),
                    MUL,
                )
                zeng = nc.gpsimd if z_on_pool else nc.vector
                zeng.tensor_tensor(a_all[:, 2 * W49:], a_all[:, 2 * W49:],
                                   s_t[:], MUL)

                # ---- tree prefix sums -> x|y|z in one tile [P, 3, G, 50]
                x_all = pout.tile([P, 3 * W50], F32, tag="x")
                nc.vector.tensor_copy(
                    _ap3(x_all, 0, [[W50, 3], [NP, G], [1, 1]]),
                    _ap3(rall, g0, [[stripe, 3], [1, G], [0, 1]]),
                )
                for (lo, hi) in LEVELS:
                    n = hi - lo
                    no = (n + 1) // 2   # odd children lo, lo+2, ...
                    ne = n // 2         # even children lo+1, lo+3, ...
                    plo = (lo - 1) // 2
                    nc.vector.tensor_tensor(
                        _ap3(x_all, lo, [[W50, 3], [NP, G], [2, no]]),
                        _ap3(x_all, plo, [[W50, 3], [NP, G], [1, no]]),
                        _ap3(a_all, lo - 1, [[W49, 3], [NL, G], [2, no]]),
                        ADD,
                    )
                    if ne:
                        nc.vector.tensor_tensor(
                            _ap3(x_all, lo + 1, [[W50, 3], [NP, G], [2, ne]]),
                            _ap3(x_all, plo, [[W50, 3], [NP, G], [1, ne]]),
                            _ap3(a_all, lo, [[W49, 3], [NL, G], [2, ne]]),
                            ADD,
                        )

                # ---- store x,y,z (split across the two HWDGE rings)
                nc.scalar.dma_start(out=xv[:, g0:g0 + G, :], in_=x_all[:, :W50])
                nc.scalar.dma_start(out=yv[:, g0:g0 + G, :],
                                    in_=x_all[:, W50:2 * W50])
                nc.sync.dma_start(out=zv[:, g0:g0 + G, :],
                                  in_=x_all[:, 2 * W50:])

                # ---- loss partial: sum w*((x-tx)^2+(y-ty)^2)
                ex = pscr.tile([P, 2 * W50], BF16, tag="ex")
                seng = nc.gpsimd if sub_on_pool else nc.vector
                seng.tensor_tensor(
                    _ap3(ex, 0, [[W50, 2], [NP, G], [1, NP]]),
                    _ap3(x_all, 0, [[W50, 2], [NP, G], [1, NP]]),
                    _ap3(txy_t, 0, [[NP, 2], [2 * NP, G], [1, NP]]),
                    SUB,
                )
                nc.vector.tensor_tensor(
                    _ap3(ex, 0, [[W50, 2], [1, W50]]),
                    _ap3(ex, 0, [[W50, 2], [1, W50]]),
                    _ap3(sw_t, 0, [[0, 2], [1, W50]]),
                    MUL,
                )
                nc.scalar.activation(out=ex[:], in_=ex[:], func=SQ,
                                     accum_out=acc_l[:, gi:gi + 1])

                # ---- reg2 partial: sum over coords of within-group diffs
                Wd = (G - 1) * NP
                wpad = -(-3 * Wd // 512) * 512
                df = pscr.tile([P, wpad], BF16, tag="df")
                if wpad > 3 * Wd:
                    nc.vector.memset(df[:, 3 * Wd:], 0.0)
                deng = nc.gpsimd if diff_on_pool else nc.vector
                deng.tensor_tensor(
                    _ap3(df, 0, [[Wd, 3], [NP, G - 1], [1, NP]]),
                    _ap3(x_all, 0, [[W50, 3], [NP, G - 1], [1, NP]]),
                    _ap3(x_all, NP, [[W50, 3], [NP, G - 1], [1, NP]]),
                    SUB,
                )
                # square in place (bf16 2x) then reduce over partitions on PE
                nc.vector.tensor_tensor(df[:, :3 * Wd], df[:, :3 * Wd],
                                        df[:, :3 * Wd], MUL)
                for off in range(0, wpad, 512):
                    nc.tensor.matmul(rps[:], ones_bf[:],
                                     df[:, off:off + 512],
                                     start=(mm_idx[0] == 0),
                                     stop=(mm_idx[0] == total_mm - 1))
                    mm_idx[0] += 1
                g0 += G

            stat_r = pone.tile([1, 512], F32, tag="statr")
            nc.vector.tensor_copy(stat_r[:], rps[:])
            nc.sync.dma_start(out=sl_o[:], in_=acc_l[:])
            nc.sync.dma_start(out=sr_o[:], in_=stat_r[:])

    if split_waits:
        _split_excess_waits(nc)
    return nc


def _get_nc():
    if "nc" not in _cache:
        _cache["nc"] = _build_bass()
    return _cache["nc"]


def _host_ref(lines, rx, ry, rz, ax, ay, az):
    """Host fallback math for a row slab: returns x, y, z (f32)."""
    L = np.exp(lines.astype(np.float64))[BONE_IDX]
    na = np.sqrt(ax.astype(np.float64) ** 2 + ay.astype(np.float64) ** 2
                 + az.astype(np.float64) ** 2) + EPS
    Pm = _path_matrix().astype(np.float64)
    dx = L * ax / na
    dy = L * ay / na
    dz = L * az / na
    x = rx + dx @ Pm.T
    y = ry + dy @ Pm.T
    z = rz + dz @ Pm.T
    return x.astype(np.float32), y.astype(np.float32), z.astype(np.float32)


def _missing_pairs():
    """reg2 pairs (r, r+1) not covered on-device."""
    if "mp" in _cache:
        return _cache["mp"]
    covered = np.zeros(T - 1, dtype=bool)
    ns = []
    g0 = 0
    for G in G_LIST:
        ns.append(np.arange(g0, g0 + G - 1))
        g0 += G
    ns = np.concatenate(ns)  # within-stripe tile indices with covered pair
    base = (np.arange(NCORES)[:, None] * RPC
            + np.arange(P)[None, :] * S).reshape(-1, 1)
    rows = (base + ns[None, :]).ravel()
    covered[rows] = True
    miss = np.nonzero(~covered)[0]
    _cache["mp"] = miss
    return miss


def kernel(**inputs):
    lines = np.asarray(inputs["lines"], dtype=np.float32)
    rootsx = np.asarray(inputs["rootsx"], dtype=np.float32)
    rootsy = np.asarray(inputs["rootsy"], dtype=np.float32)
    rootsz = np.asarray(inputs["rootsz"], dtype=np.float32)
    anglesx = np.asarray(inputs["anglesx"], dtype=np.float32)
    anglesy = np.asarray(inputs["anglesy"], dtype=np.float32)
    anglesz = np.asarray(inputs["anglesz"], dtype=np.float32)
    tarx = np.asarray(inputs["tarx_values"], dtype=np.float32)
    tary = np.asarray(inputs["tary_values"], dtype=np.float32)
    wv = np.asarray(inputs["w_values"], dtype=np.float32)

    L_limb = np.exp(lines)[BONE_IDX]  # (49,) f32
    ltile = np.ascontiguousarray(
        np.tile(L_limb, max(G_LIST))[None, :]).astype(np.float32)
    bf16 = mybir.dt.np(BF16)
    txy = np.concatenate([tarx, tary], axis=1).astype(bf16)  # (T, 100)
    sw = np.sqrt(wv).astype(bf16)

    nc = _get_nc()
    in_maps = []
    for k in range(NCORES):
        a, b = k * RPC, (k + 1) * RPC
        in_maps.append({
            "ax": np.ascontiguousarray(anglesx[a:b]),
            "ay": np.ascontiguousarray(anglesy[a:b]),
            "az": np.ascontiguousarray(anglesz[a:b]),
            "txy": np.ascontiguousarray(txy[a:b]),
            "sw": np.ascontiguousarray(sw[a:b]),
            "rx": np.ascontiguousarray(rootsx[a:b]),
            "ry": np.ascontiguousarray(rootsy[a:b]),
            "rz": np.ascontiguousarray(rootsz[a:b]),
            "ltile": ltile,
        })

    res = run_bass_kernel_spmd(nc, in_maps, core_ids=list(range(NCORES)),
                               trace=TRACE)
    _cache["last"] = res
    outs = res.results

    x = np.empty((T, NP), dtype=np.float32)
    y = np.empty((T, NP), dtype=np.float32)
    z = np.empty((T, NP), dtype=np.float32)
    loss_sum = 0.0
    reg2_sum = 0.0
    for k in range(NCORES):
        a, b = k * RPC, (k + 1) * RPC
        x[a:b] = outs[k]["x"]
        y[a:b] = outs[k]["y"]
        z[a:b] = outs[k]["z"]
        loss_sum += float(outs[k]["sloss"].astype(np.float64).sum())
        reg2_sum += float(outs[k]["sreg2"].astype(np.float64).sum())

    # host tail (last 320 rows)
    if MAIN < T:
        sl = slice(MAIN, T)
        xt, yt, zt = _host_ref(lines, rootsx[sl], rootsy[sl], rootsz[sl],
                               anglesx[sl], anglesy[sl], anglesz[sl])
        x[sl], y[sl], z[sl] = xt, yt, zt
        ex = x[sl].astype(np.float64) - tarx[sl]
        ey = y[sl].astype(np.float64) - tary[sl]
        loss_sum += float((wv[sl] * (ex * ex + ey * ey)).sum())

    # host reg2 for pairs not covered on-device
    miss = _missing_pairs()
    for arr in (x, y, z):
        d = arr[miss].astype(np.float64) - arr[miss + 1]
        reg2_sum += float((d * d).sum())

    reg1 = float(np.exp(lines.astype(np.float64)).sum())
    total = (loss_sum / (T * NP) + REG_RATES[0] * reg1
             + REG_RATES[1] * reg2_sum / ((T - 1) * NP))
    return x, y, z, np.float32(total)


# revision 36
# speedup vs baseline: 2.0359x; 1.6108x over previous
"""Trainium2 Bass kernel for nn_BackpropagationBasedFiltering.

Computation (see reference):
  L = exp(lines)[bone_idx]                       (49,)
  norm = sqrt(ax^2+ay^2+az^2) + 1e-10            (T,49)
  d{x,y,z} = L * a{x,y,z} / norm                 (T,49)
  x = rootsx + dx @ PATH_T                       (T,50)   (binary-tree path sums)
  loss = sum(w*(x-tx)^2 + w*(y-ty)^2) / (T*50)
  reg1 = sum(exp(lines))
  reg2 = sum((x[:-1]-x[1:])^2 + ... y,z ...) / ((T-1)*50)
  returns (x, y, z, loss + 0.001*reg1 + 0.1*reg2)

Strategy: data-parallel over T on 8 cores, 24960 rows/core (128 partitions x
stripes of 195 consecutive rows). The path-matrix product is evaluated as a
level-vectorized binary-tree prefix sum on the vector engine with strided
access patterns (no PE transposes needed). Loss/reg2 reduced on-chip
(ACT square+accum, DVE tensor_tensor_reduce); scalar partials combined on
host. The last 320 rows and the stripe/core-boundary reg2 pairs are computed
on the host (0.16% of the work).
"""

import sys

sys.path.insert(0, "/opt/trn_rl_repo")

import numpy as np

import bass_rust
import concourse.bass as bass
import concourse.tile as tile_mod
from concourse import mybir
from concourse.bass_utils import run_bass_kernel_spmd
from concourse.tile import TileContext


def _drain_and_barrier_split(self, tick_clock, wait_clock):
    """Tail drain with one sem-wait per instruction: the walrus build here
    rejects CTRL instructions carrying >4 sync waits, and Tile's stock tail
    drain waits on every tracked semaphore at once."""
    nc = self.nc
    drain_inst = nc.sync.drain()
    wait_clock.add_sem_waits(
        drain_inst.ins, tile_mod.ScopedClock({None: tick_clock.global_clock})
    )
    si = drain_inst.ins.sync_info
    waits = list(si.on_wait or [])
    if len(waits) > 1:
        si.on_wait = waits[:1]
        for w in waits[1:]:
            nop = nc.sync.nop(nofuse=True, hint="tail_wait_split")
            nsi = nop.ins.sync_info
            if nsi is None:
                nop.ins.sync_info = bass_rust.SyncInfo(on_wait=[w], on_update=[])
            else:
                nsi.on_wait = [w]
    nc.all_engine_barrier()
    popped = nc._tile_sem_poison_stack.pop()
    assert popped is self._sem_poison
    nc.clear_and_free_semaphores(list(self.sems.allocated().values()))
    nc.all_engine_barrier()


TileContext._drain_and_barrier = _drain_and_barrier_split

_MAX_WAITS = 1  # this walrus rejects instructions with more sync waits


def _split_excess_waits(nc):
    """Hoist excess sem-waits onto same-engine NOPs emitted just before the
    offending instruction (engines execute their stream in order, so the
    semantics are identical)."""
    uid = [0]
    for f in nc.m.functions:
        for bb in f.blocks:
            insts = bb.instructions
            rebuilt = []
            changed = False
            for inst in insts:
                si = getattr(inst, "sync_info", None)
                ow = list(si.on_wait) if si is not None and si.on_wait else []
                if len(ow) > _MAX_WAITS:
                    keep = ow[-_MAX_WAITS:]
                    excess = ow[:-_MAX_WAITS]
                    for j in range(0, len(excess), _MAX_WAITS):
                        nop = mybir.InstNoOp(
                            name=f"waitsplit-{uid[0]}", ins=[], outs=[])
                        uid[0] += 1
                        nop.engine = inst.engine
                        nop.sync_info = bass_rust.SyncInfo(
                            on_wait=excess[j:j + _MAX_WAITS], on_update=[])
                        rebuilt.append(nop)
                    si.on_wait = keep
                    changed = True
                rebuilt.append(inst)
            if changed:
                del insts[:]
                for inst in rebuilt:
                    insts.append(inst)

# ---------------------------------------------------------------- constants
T = 200000
NL = 49          # limbs
NP = 50          # points
NB = 20          # bones
EPS = 1e-10
REG_RATES = (0.001, 0.1)
NCORES = 8
P = 128          # partitions
S = 195          # stripe length (rows per partition per core)
RPC = P * S      # rows per core = 24960
MAIN = NCORES * RPC  # 199680
# groups of tiles processed per instruction batch (sum == S)
G_LIST = [33, 33, 33, 32, 32, 32]
NGRP = len(G_LIST)

STRUCTURE = [((b - 1) // 2, b, (b - 1) % NB) for b in range(1, NP)]
BONE_IDX = np.array([l for (_, _, l) in STRUCTURE], dtype=np.int64)
# tree levels: (lo, hi) node ranges; children of p are 2p+1, 2p+2
LEVELS = [(1, 3), (3, 7), (7, 15), (15, 31), (31, 50)]

F32 = mybir.dt.float32
BF16 = mybir.dt.bfloat16

TRACE = False  # test harness can flip this to collect an NTFF profile
_cache = {}


def _path_matrix():
    Pm = np.zeros((NP, NL), dtype=np.float32)
    for i, (a, b, _) in enumerate(STRUCTURE):
        Pm[b] = Pm[a]
        Pm[b, i] = 1.0
    return Pm


def _ap3(tile, col_off, dims):
    """AP into `tile` (full partition dim) with custom free dims, offset in
    elements from the tile's first column."""
    t = tile[:]
    return bass.AP(tensor=t.tensor, offset=t.offset + col_off, ap=[t.ap[0]] + dims)


def _build_bass(stripe=S, g_list=G_LIST, split_waits=True, epochs=1,
                z_on_pool=True, diff_on_pool=False, sub_on_pool=True,
                tree_odd_pool=False, sadd2_pool=False):
    """Build the per-core bass program. All 8 cores run it SPMD on their
    own 24960-row slice.

    Combined-tile layout: the three coordinates live in one SBUF tile
    (coord-major) so each elementwise/tree step is a single wide
    instruction with strided access patterns. Loss uses bf16 scratch with
    host-precomputed sqrt(w); reductions ride the ACT square's accum_out.
    """
    nc = bass.Bass()
    rows = P * stripe
    ngrp = len(g_list)

    ax_e = nc.dram_tensor("ax", [rows, NL], F32, kind="ExternalInput")
    ay_e = nc.dram_tensor("ay", [rows, NL], F32, kind="ExternalInput")
    az_e = nc.dram_tensor("az", [rows, NL], F32, kind="ExternalInput")
    txy_e = nc.dram_tensor("txy", [rows, 2 * NP], BF16, kind="ExternalInput")
    sw_e = nc.dram_tensor("sw", [rows, NP], BF16, kind="ExternalInput")
    rx_e = nc.dram_tensor("rx", [rows, 1], F32, kind="ExternalInput")
    ry_e = nc.dram_tensor("ry", [rows, 1], F32, kind="ExternalInput")
    rz_e = nc.dram_tensor("rz", [rows, 1], F32, kind="ExternalInput")
    lt_e = nc.dram_tensor("ltile", [1, max(g_list) * NL], F32, kind="ExternalInput")

    x_o = nc.dram_tensor("x", [rows, NP], F32, kind="ExternalOutput")
    y_o = nc.dram_tensor("y", [rows, NP], F32, kind="ExternalOutput")
    z_o = nc.dram_tensor("z", [rows, NP], F32, kind="ExternalOutput")
    sl_o = nc.dram_tensor("sloss", [P, ngrp], F32, kind="ExternalOutput")
    sr_o = nc.dram_tensor("sreg2", [1, 512], F32, kind="ExternalOutput")

    # DRAM views: row (p*stripe + n)  ->  partition p, tile n
    axv = ax_e[:].rearrange("(p s) l -> p s l", p=P)
    ayv = ay_e[:].rearrange("(p s) l -> p s l", p=P)
    azv = az_e[:].rearrange("(p s) l -> p s l", p=P)
    txyv = txy_e[:].rearrange("(p s) l -> p s l", p=P)
    swv = sw_e[:].rearrange("(p s) l -> p s l", p=P)
    xv = x_o[:].rearrange("(p s) l -> p s l", p=P)
    yv = y_o[:].rearrange("(p s) l -> p s l", p=P)
    zv = z_o[:].rearrange("(p s) l -> p s l", p=P)
    rxv = rx_e[:].rearrange("(p s) 1 -> p s", p=P)
    ryv = ry_e[:].rearrange("(p s) 1 -> p s", p=P)
    rzv = rz_e[:].rearrange("(p s) 1 -> p s", p=P)

    gmax = max(g_list)
    ADD = mybir.AluOpType.add
    SUB = mybir.AluOpType.subtract
    MUL = mybir.AluOpType.mult
    SQ = mybir.ActivationFunctionType.Square

    with TileContext(nc) as tc:
        with (
            tc.tile_pool(name="pa", bufs=2) as pa,
            tc.tile_pool(name="ptar", bufs=2) as ptar,
            tc.tile_pool(name="pout", bufs=2) as pout,
            tc.tile_pool(name="psq", bufs=2) as psq,
            tc.tile_pool(name="ps2", bufs=3) as ps2,
            tc.tile_pool(name="pscr", bufs=2) as pscr,
            tc.tile_pool(name="pone", bufs=1) as pone,
            tc.tile_pool(name="pps", bufs=1, space="PSUM") as pps,
        ):
            # one-time loads
            lt = pone.tile([P, gmax * NL], F32, tag="lt")
            ltap = lt_e[:]
            nc.gpsimd.dma_start(
                out=lt[:],
                in_=bass.AP(tensor=ltap.tensor, offset=ltap.offset,
                            ap=[[0, P], [1, gmax * NL]]),
            )
            rall = pone.tile([P, 3 * stripe], F32, tag="rall")
            for ci, rv in enumerate((rxv, ryv, rzv)):
                nc.sync.dma_start(out=rall[:, ci * stripe:(ci + 1) * stripe],
                                  in_=rv)
            acc_l = pone.tile([P, ngrp], F32, tag="accl")
            ones_bf = pone.tile([P, 1], BF16, tag="ones")
            nc.vector.memset(ones_bf[:], 1.0)
            rps = pps.tile([1, 512], F32, tag="rps")
            total_mm = epochs * sum(-(-(3 * (G - 1) * NP) // 512)
                                    for G in g_list)  # padded chunks per group
            mm_idx = [0]

            for _ep in range(epochs):
              g0 = 0
              for gi, G in enumerate(g_list):
                W49 = G * NL
                W50 = G * NP
                # ---- loads
                a_all = pa.tile([P, 3 * W49], F32, tag="a")
                for ci, v in enumerate((axv, ayv, azv)):
                    nc.sync.dma_start(out=a_all[:, ci * W49:(ci + 1) * W49],
                                      in_=v[:, g0:g0 + G, :])
                txy_t = ptar.tile([P, 2 * W50], BF16, tag="txy")
                nc.sync.dma_start(out=txy_t[:], in_=txyv[:, g0:g0 + G, :])
                sw_t = ptar.tile([P, W50], BF16, tag="sw")
                nc.sync.dma_start(out=sw_t[:], in_=swv[:, g0:g0 + G, :])

                # ---- norm: s = ax^2+ay^2+az^2 ; inv2 = L * s^-0.5
                sq_all = psq.tile([P, 3 * W49], F32, tag="sq")
                nc.scalar.square(sq_all[:], a_all[:])
                s_t = ps2.tile([P, W49], F32, tag="s")
                nc.vector.tensor_tensor(s_t[:], sq_all[:, :W49],
                                        sq_all[:, W49:2 * W49], ADD)
                (nc.gpsimd if sadd2_pool else nc.vector).tensor_tensor(
                    s_t[:], s_t[:], sq_all[:, 2 * W49:], ADD)
                # 1/sqrt(s) = exp(-0.5*ln(s)); ACT Rsqrt/Reciprocal are
                # banned for accuracy; Ln/Exp share one table set.
                nc.scalar.activation(s_t[:], s_t[:],
                                     mybir.ActivationFunctionType.Ln)
                nc.scalar.activation(s_t[:], s_t[:],
                                     mybir.ActivationFunctionType.Exp,
                                     scale=-0.5)
                nc.vector.tensor_tensor(s_t[:], s_t[:], lt[:, :W49], MUL)
                # d = a * inv2 (in place; broadcast inv2 across coords)
                nc.vector.tensor_tensor(
                    _ap3(a_all, 0, [[W49, 2], [1, W49]]),
                    _ap3(a_all, 0, [[W49, 2], [1, W49]]),
                    _ap3(s_t, 0, [[0, 2], [1, W49]]),
                    MUL,
                )
                zeng = nc.gpsimd if z_on_pool else nc.vector
                zeng.tensor_tensor(a_all[:, 2 * W49:], a_all[:, 2 * W49:],
                                   s_t[:], MUL)

                # ---- tree prefix sums -> x|y|z in one tile [P, 3, G, 50]
                x_all = pout.tile([P, 3 * W50], F32, tag="x")
                nc.vector.tensor_copy(
                    _ap3(x_all, 0, [[W50, 3], [NP, G], [1, 1]]),
                    _ap3(rall, g0, [[stripe, 3], [1, G], [0, 1]]),
                )
                oeng = nc.gpsimd if tree_odd_pool else nc.vector
                for (lo, hi) in LEVELS:
                    n = hi - lo
                    no = (n + 1) // 2   # odd children lo, lo+2, ...
                    ne = n // 2         # even children lo+1, lo+3, ...
                    plo = (lo - 1) // 2
                    oeng.tensor_tensor(
                        _ap3(x_all, lo, [[W50, 3], [NP, G], [2, no]]),
                        _ap3(x_all, plo, [[W50, 3], [NP, G], [1, no]]),
                        _ap3(a_all, lo - 1, [[W49, 3], [NL, G], [2, no]]),
                        ADD,
                    )
                    if ne:
                        nc.vector.tensor_tensor(
                            _ap3(x_all, lo + 1, [[W50, 3], [NP, G], [2, ne]]),
                            _ap3(x_all, plo, [[W50, 3], [NP, G], [1, ne]]),
                            _ap3(a_all, lo, [[W49, 3], [NL, G], [2, ne]]),
                            ADD,
                        )

                # ---- store x,y,z (split across the two HWDGE rings)
                nc.scalar.dma_start(out=xv[:, g0:g0 + G, :], in_=x_all[:, :W50])
                nc.scalar.dma_start(out=yv[:, g0:g0 + G, :],
                                    in_=x_all[:, W50:2 * W50])
                nc.sync.dma_start(out=zv[:, g0:g0 + G, :],
                                  in_=x_all[:, 2 * W50:])

                # ---- loss partial: sum w*((x-tx)^2+(y-ty)^2)
                ex = pscr.tile([P, 2 * W50], BF16, tag="ex")
                seng = nc.gpsimd if sub_on_pool else nc.vector
                seng.tensor_tensor(
                    _ap3(ex, 0, [[W50, 2], [NP, G], [1, NP]]),
                    _ap3(x_all, 0, [[W50, 2], [NP, G], [1, NP]]),
                    _ap3(txy_t, 0, [[NP, 2], [2 * NP, G], [1, NP]]),
                    SUB,
                )
                nc.vector.tensor_tensor(
                    _ap3(ex, 0, [[W50, 2], [1, W50]]),
                    _ap3(ex, 0, [[W50, 2], [1, W50]]),
                    _ap3(sw_t, 0, [[0, 2], [1, W50]]),
                    MUL,
                )
                nc.scalar.activation(out=ex[:], in_=ex[:], func=SQ,
                                     accum_out=acc_l[:, gi:gi + 1])

                # ---- reg2 partial: sum over coords of within-group diffs
                Wd = (G - 1) * NP
                wpad = -(-3 * Wd // 512) * 512
                df = pscr.tile([P, wpad], BF16, tag="df")
                if wpad > 3 * Wd:
                    nc.vector.memset(df[:, 3 * Wd:], 0.0)
                deng = nc.gpsimd if diff_on_pool else nc.vector
                deng.tensor_tensor(
                    _ap3(df, 0, [[Wd, 3], [NP, G - 1], [1, NP]]),
                    _ap3(x_all, 0, [[W50, 3], [NP, G - 1], [1, NP]]),
                    _ap3(x_all, NP, [[W50, 3], [NP, G - 1], [1, NP]]),
                    SUB,
                )
                # square in place (bf16 2x) then reduce over partitions on PE
                nc.vector.tensor_tensor(df[:, :3 * Wd], df[:, :3 * Wd],
                                        df[:, :3 * Wd], MUL)
                for off in range(0, wpad, 512):
                    nc.tensor.matmul(rps[:], ones_bf[:],
                                     df[:, off:off + 512],
                                     start=(mm_idx[0] == 0),
                                     stop=(mm_idx[0] == total_mm - 1))
                    mm_idx[0] += 1
                g0 += G

            stat_r = pone.tile([1, 512], F32, tag="statr")
            nc.vector.tensor_copy(stat_r[:], rps[:])
            nc.sync.dma_start(out=sl_o[:], in_=acc_l[:])
            nc.sync.dma_start(out=sr_o[:], in_=stat_r[:])

    if split_waits:
        _split_excess_waits(nc)
    return nc


def _get_nc():
    if "nc" not in _cache:
        _cache["nc"] = _build_bass()
    return _cache["nc"]


def _host_ref(lines, rx, ry, rz, ax, ay, az):
    """Host fallback math for a row slab: returns x, y, z (f32)."""
    L = np.exp(lines.astype(np.float64))[BONE_IDX]
    na = np.sqrt(ax.astype(np.float64) ** 2 + ay.astype(np.float64) ** 2
                 + az.astype(np.float64) ** 2) + EPS
    Pm = _path_matrix().astype(np.float64)
    dx = L * ax / na
    dy = L * ay / na
    dz = L * az / na
    x = rx + dx @ Pm.T
    y = ry + dy @ Pm.T
    z = rz + dz @ Pm.T
    return x.astype(np.float32), y.astype(np.float32), z.astype(np.float32)


def _missing_pairs():
    """reg2 pairs (r, r+1) not covered on-device."""
    if "mp" in _cache:
        return _cache["mp"]
    covered = np.zeros(T - 1, dtype=bool)
    ns = []
    g0 = 0
    for G in G_LIST:
        ns.append(np.arange(g0, g0 + G - 1))
        g0 += G
    ns = np.concatenate(ns)  # within-stripe tile indices with covered pair
    base = (np.arange(NCORES)[:, None] * RPC
            + np.arange(P)[None, :] * S).reshape(-1, 1)
    rows = (base + ns[None, :]).ravel()
    covered[rows] = True
    miss = np.nonzero(~covered)[0]
    _cache["mp"] = miss
    return miss


def kernel(**inputs):
    lines = np.asarray(inputs["lines"], dtype=np.float32)
    rootsx = np.asarray(inputs["rootsx"], dtype=np.float32)
    rootsy = np.asarray(inputs["rootsy"], dtype=np.float32)
    rootsz = np.asarray(inputs["rootsz"], dtype=np.float32)
    anglesx = np.asarray(inputs["anglesx"], dtype=np.float32)
    anglesy = np.asarray(inputs["anglesy"], dtype=np.float32)
    anglesz = np.asarray(inputs["anglesz"], dtype=np.float32)
    tarx = np.asarray(inputs["tarx_values"], dtype=np.float32)
    tary = np.asarray(inputs["tary_values"], dtype=np.float32)
    wv = np.asarray(inputs["w_values"], dtype=np.float32)

    L_limb = np.exp(lines)[BONE_IDX]  # (49,) f32
    ltile = np.ascontiguousarray(
        np.tile(L_limb, max(G_LIST))[None, :]).astype(np.float32)
    bf16 = mybir.dt.np(BF16)
    txy = np.concatenate([tarx, tary], axis=1).astype(bf16)  # (T, 100)
    sw = np.sqrt(wv).astype(bf16)

    nc = _get_nc()
    in_maps = []
    for k in range(NCORES):
        a, b = k * RPC, (k + 1) * RPC
        in_maps.append({
            "ax": np.ascontiguousarray(anglesx[a:b]),
            "ay": np.ascontiguousarray(anglesy[a:b]),
            "az": np.ascontiguousarray(anglesz[a:b]),
            "txy": np.ascontiguousarray(txy[a:b]),
            "sw": np.ascontiguousarray(sw[a:b]),
            "rx": np.ascontiguousarray(rootsx[a:b]),
            "ry": np.ascontiguousarray(rootsy[a:b]),
            "rz": np.ascontiguousarray(rootsz[a:b]),
            "ltile": ltile,
        })

    res = run_bass_kernel_spmd(nc, in_maps, core_ids=list(range(NCORES)),
                               trace=TRACE)
    _cache["last"] = res
    outs = res.results

    x = np.empty((T, NP), dtype=np.float32)
    y = np.empty((T, NP), dtype=np.float32)
    z = np.empty((T, NP), dtype=np.float32)
    loss_sum = 0.0
    reg2_sum = 0.0
    for k in range(NCORES):
        a, b = k * RPC, (k + 1) * RPC
        x[a:b] = outs[k]["x"]
        y[a:b] = outs[k]["y"]
        z[a:b] = outs[k]["z"]
        loss_sum += float(outs[k]["sloss"].astype(np.float64).sum())
        reg2_sum += float(outs[k]["sreg2"].astype(np.float64).sum())

    # host tail (last 320 rows)
    if MAIN < T:
        sl = slice(MAIN, T)
        xt, yt, zt = _host_ref(lines, rootsx[sl], rootsy[sl], rootsz[sl],
                               anglesx[sl], anglesy[sl], anglesz[sl])
        x[sl], y[sl], z[sl] = xt, yt, zt
        ex = x[sl].astype(np.float64) - tarx[sl]
        ey = y[sl].astype(np.float64) - tary[sl]
        loss_sum += float((wv[sl] * (ex * ex + ey * ey)).sum())

    # host reg2 for pairs not covered on-device
    miss = _missing_pairs()
    for arr in (x, y, z):
        d = arr[miss].astype(np.float64) - arr[miss + 1]
        reg2_sum += float((d * d).sum())

    reg1 = float(np.exp(lines.astype(np.float64)).sum())
    total = (loss_sum / (T * NP) + REG_RATES[0] * reg1
             + REG_RATES[1] * reg2_sum / ((T - 1) * NP))
    return x, y, z, np.float32(total)
